# revision 28
# baseline (speedup 1.0000x reference)
"""Self-contained Trainium2 Bass kernel for single-head full-dim attention.

Reference computation (fp32 jax):
    q  = x @ Wq                      # [B, Nq, D]
    kv = y @ Wkv                     # [B, Nkv, 2D] -> k, v
    attn = softmax(q * D^-0.5 @ k^T) # [B, Nq, Nkv]
    out  = attn @ v                  # [B, Nq, D]
with B=4, Nq=Nkv=2048, D=1024.

Distribution: data parallel over 8 NeuronCores, shard = (batch b,
kv-half s).  Each core computes q for ALL 2048 queries of its batch
(cheap, duplicated across the pair), K/V for its 1024 keys, the
2048x1024 exp-score block, and the UNNORMALIZED output block
out'_s = exp(S_s) @ v_s plus the partial softmax denominator
Z_s = sum_k exp(S_s).  The host combines the two halves:
out = (out'_0 + out'_1) / (Z_0 + Z_1).  This avoids both collectives
and the (2x more expensive) duplicated K/V compute of a query-sharded
layout.

Layout trick: everything on-chip is computed transposed
([feature, token]) so the TensorEngine can contract along partitions
without any on-chip transposes.  The host pre-transposes and packs each
input into the EXACT [128, N] p-major SBUF image, so every input DMA is
a single fully-dense descriptor (16-32KB contiguous per partition ->
peak HBM read bw; strided 256B-row layouts measured only ~217 GB/s vs
~400 GB/s dense).  Issue order = criticality: wk slabs 0-2, yt (P2's
gate), wk slabs 3-7, wv (P3), xt + wq (P1, needed ~60us in).  The
D^-0.5 scale is folded into Wq; all matmul operands are bf16 (fp32
PSUM accumulation).  Softmax uses exp without max-subtraction (scores
~ N(0,1) by construction; fp32 exp is safe) on the scalar engine; Z is
a ones-vector matmul row, shipped as [1, NQ].
"""

import numpy as np
import ml_dtypes

import concourse.bass as bass
import concourse.mybir as mybir
import concourse.tile as tile
from concourse.bass import ds
from concourse.bass_utils import run_bass_kernel_spmd

DIM = 1024
B = 4
NQ = 2048
NKV = 2048
N_CORES = 8
NKV_SHARD = 1024  # keys per core
WARMUP = 22  # HAM warm-up matmuls; ends ~when yt lands (~16us)

BF16 = mybir.dt.bfloat16
F32 = mybir.dt.float32
NP_BF16 = ml_dtypes.bfloat16


def _split_sync_waits(nc, max_waits: int = 1):
    """walrus in this toolchain rejects instructions carrying more than one
    sem wait ("Too many sync wait commands").  Hoist extra waits onto
    preceding same-engine NOPs: the engine dispatches in order, so waiting
    just before the instruction is semantically identical (at worst it
    delays issue slightly)."""
    import bass_rust as _bass_rust

    for f in nc.m.functions:
        for bb in f.blocks:
            insts = list(bb.instructions)
            out = []
            changed = False
            for inst in insts:
                si = getattr(inst, "sync_info", None)
                waits = list(si.on_wait) if si is not None and si.on_wait else []
                if len(waits) > max_waits:
                    changed = True
                    extra, keep = waits[:-max_waits], waits[-max_waits:]
                    for k in range(0, len(extra), max_waits):
                        nop = mybir.InstNoOp(
                            name=f"{inst.name}_sw{k}", engine=inst.engine,
                            ins=[], outs=[],
                        )
                        nop.sync_info = _bass_rust.SyncInfo(
                            on_wait=extra[k : k + max_waits], on_update=[]
                        )
                        out.append(nop)
                    si.on_wait = keep
                    inst.sync_info = si
                out.append(inst)
            if changed:
                bb.instructions = out


def build_attention_nc():
    """Build the per-core Bass graph (identical on all 8 cores)."""
    nc = bass.Bass()

    # DRAM parameters: host-packed p-major SBUF images (see module doc).
    # wk/wq slab j lives at columns [j*1024, (j+1)*1024), entry (p, j*1024
    # + c*128 + m) = W[c*128 + p, j*128 + m]; yt/xt/wv chunk c at columns
    # [c*N, (c+1)*N), entry (p, c*N + n) = T[c*128 + p, n].
    wk0_d = nc.declare_dram_parameter("wk0", [128, 1024], BF16, isOutput=False)
    yt_d = nc.declare_dram_parameter("yt", [128, 8 * NKV_SHARD], BF16, isOutput=False)
    wk17_d = nc.declare_dram_parameter("wk17", [128, 7 * 1024], BF16, isOutput=False)
    wv_d = nc.declare_dram_parameter("wv", [128, 8 * 1024], BF16, isOutput=False)
    xt_d = nc.declare_dram_parameter("xt", [128, 8 * NQ], BF16, isOutput=False)
    wq_d = nc.declare_dram_parameter("wq", [128, 4 * 1024], BF16, isOutput=False)
    pidx_d = nc.declare_dram_parameter("pidx", [128, 1], mybir.dt.int32, isOutput=False)
    sidx_d = nc.declare_dram_parameter("sidx", [128, 1], mybir.dt.int32, isOutput=False)
    out_d = nc.declare_dram_parameter("out", [NQ, DIM], F32, isOutput=True)
    z_d = nc.declare_dram_parameter("zout", [1, NQ], F32, isOutput=True)

    with tile.TileContext(nc) as tc:
        # Long-lived pool: on-chip intermediates live to the end.
        L = tc.alloc_tile_pool(name="L", bufs=1)
        pm = tc.alloc_tile_pool(name="pm", bufs=1, space="PSUM")
        # Transient input pools, released once consumed (LIFO: t1 first).
        t2 = tc.alloc_tile_pool(name="t2", bufs=1)
        t1 = tc.alloc_tile_pool(name="t1", bufs=1)

        # ---- HAM warm-up: dummy matmuls on a zeroed scratch tile run
        # during the otherwise-idle input-DMA window, flipping the PE clock
        # gate to 8/8 (2.4GHz) before the first real matmul arrives.
        ws = t1.tile([128, 512], BF16, name="warm", tag="warm", bufs=1)
        nc.gpsimd.memset(ws[:], 0.0)
        wps = pm.tile([128, 512], F32, name="wps", tag="z", bufs=2)
        for w in range(WARMUP):
            nc.tensor.matmul(
                wps[:], lhsT=ws[:, 0:128], rhs=ws[:],
                start=(w == 0), stop=(w == WARMUP - 1),
            )

        # ---- Input DMAs: one dense descriptor per tensor, issue order =
        # transfer order = criticality order.
        wk0 = t2.tile([128, 1, 1024], BF16, name="wk0", bufs=1)
        nc.sync.dma_start(out=wk0[:], in_=wk0_d.rearrange("p (s n) -> p s n", s=1))
        yt = t2.tile([128, 8, NKV_SHARD], BF16, name="yt", bufs=1)
        nc.sync.dma_start(out=yt[:], in_=yt_d.rearrange("p (c n) -> p c n", c=8))
        wk17 = t2.tile([128, 7, 1024], BF16, name="wk17", bufs=1)
        nc.sync.dma_start(out=wk17[:], in_=wk17_d.rearrange("p (s n) -> p s n", s=7))
        wq = t1.tile([128, 4, 1024], BF16, name="wq", bufs=1)
        nc.sync.dma_start(out=wq[:], in_=wq_d.rearrange("p (s n) -> p s n", s=4))
        xt = t1.tile([128, 8, NQ], BF16, name="xt", bufs=1)
        nc.sync.dma_start(out=xt[:], in_=xt_d.rearrange("p (c n) -> p c n", c=8))
        wv = t2.tile([128, 8, 1024], BF16, name="wv", bufs=1)
        nc.sync.dma_start(out=wv[:], in_=wv_d.rearrange("p (c n) -> p c n", c=8))
        pidx = t1.tile([128, 1], mybir.dt.int32, name="pidx", bufs=1)
        nc.sync.dma_start(out=pidx[:], in_=pidx_d[:])
        sidx = t1.tile([128, 1], mybir.dt.int32, name="sidx", bufs=1)
        nc.sync.dma_start(out=sidx[:], in_=sidx_d[:])

        # ---- P2: kT[do, nkv] = sum_di Wk[di, do] * yT[di, nkv] ----------
        kt = [L.tile([128, NKV_SHARD], BF16, name=f"kt{j}", tag="kt", bufs=8) for j in range(8)]
        for j in range(8):
            wkt = wk0 if j < 1 else wk17
            jj = j if j < 1 else j - 1
            for q in range(2):  # nkv 512-chunk
                ps = pm.tile([128, 512], F32, name=f"psk{j}_{q}", tag="mm", bufs=6)
                for c in range(8):
                    nc.tensor.matmul(
                        ps[:],
                        lhsT=wkt[:, jj, ds(c * 128, 128)],
                        rhs=yt[:, c, ds(q * 512, 512)],
                        start=(c == 0),
                        stop=(c == 7),
                    )
                nc.any.tensor_copy(kt[j][:, ds(q * 512, 512)], ps[:])

        # ---- P1 (half): this core computes its 4 local wq slabs; the pair
        # exchanges halves via a pairwise AllGather through DRAM bounce
        # buffers while P3 keeps the PE busy.  Host permutes wk slabs per
        # core parity so kt[j] always matches qt slab j (sum over slabs is
        # order-invariant).
        qt_all = L.tile([128, 8 * NQ], BF16, name="qt_all", bufs=1)
        for j in range(4):  # local d_out slab
            for q in range(4):  # nq 512-chunk
                ps = pm.tile([128, 512], F32, name=f"psq{j}_{q}", tag="mm", bufs=6)
                for c in range(8):  # d_in chunk (contraction)
                    nc.tensor.matmul(
                        ps[:],
                        lhsT=wq[:, j, ds(c * 128, 128)],
                        rhs=xt[:, c, ds(q * 512, 512)],
                        start=(c == 0),
                        stop=(c == 7),
                    )
                nc.any.tensor_copy(qt_all[:, ds(j * NQ + q * 512, 512)], ps[:])

        # 8-core AllGather with a SHARED output buffer (the >4-core fast
        # path: each core writes its 2MB half once into the chip-shared
        # region, one rendezvous).  Each core then gathers back only its
        # PEER's 128 rows via an indirect DMA whose row indices are a
        # host-supplied per-core input (walrus cannot encode register-offset
        # direct DMAs).  kt is host-permuted to local slab order so the
        # kt[j]*qt[j] pairing stays SPMD-uniform.
        qshare = nc.dram_tensor(
            "qshare", [8 * 128 + 1, 4 * NQ], BF16, kind="Internal",
            addr_space="Shared",
        )
        nc.gpsimd.indirect_dma_start(
            out=qshare[:],
            out_offset=bass.IndirectOffsetOnAxis(ap=sidx[:, :1], axis=0),
            in_=qt_all[:, ds(0, 2 * NQ)],
            in_offset=None,
        )
        nc.gpsimd.indirect_dma_start(
            out=qshare[:],
            out_offset=bass.IndirectOffsetOnAxis(ap=sidx[:, :1], axis=0),
            in_=qt_all[:, ds(2 * NQ, 2 * NQ)],
            in_offset=None,
            element_offset=2 * NQ,
        )
        # 256-byte 8-core AllReduce used purely as a barrier.  Its input
        # chain starts with a probe READ of qshare (conservative shadow dep
        # -> runs after this core's scatter), bounced through a Local DRAM
        # tile (collectives cannot read Shared).  Rank arrival therefore
        # implies that core's scatter completed, so cc-done means every
        # core's half is visible.  The cc writes back INTO qshare (row
        # 1024), so the gather below is shadow-ordered after the cc.
        qd = tc.alloc_tile_pool(name="dram", bufs=1, space="DRAM")
        bar_in = qd.tile([1, 128], BF16, name="bar_in", bufs=1)
        nc.gpsimd.dma_start(out=bar_in[:], in_=qshare[0:1, 0:128])
        nc.gpsimd.collective_compute(
            "AllReduce",
            mybir.AluOpType.add,
            replica_groups=[[0, 1, 2, 3, 4, 5, 6, 7]],
            ins=[bar_in.opt()],
            outs=[qshare[1024:1025, 0:128]],
        )
        nc.gpsimd.indirect_dma_start(
            out=qt_all[:, ds(4 * NQ, 4 * NQ)],
            out_offset=None,
            in_=qshare[:],
            in_offset=bass.IndirectOffsetOnAxis(ap=pidx[:, :1], axis=0),
        )
        qd.release()
        t1.release()

        # ---- P3: v[nkv, do] = sum_di yT[di, nkv] * Wv[di, do] -----------
        vt = [L.tile([128, DIM], BF16, name=f"v{i}", tag="v", bufs=8) for i in range(8)]
        for i in range(8):  # nkv 128-tile
            for d in range(2):  # d_out 512-chunk
                ps = pm.tile([128, 512], F32, name=f"psv{i}_{d}", tag="mm", bufs=6)
                for c in range(8):
                    nc.tensor.matmul(
                        ps[:],
                        lhsT=yt[:, c, ds(i * 128, 128)],
                        rhs=wv[:, c, ds(d * 512, 512)],
                        start=(c == 0),
                        stop=(c == 7),
                    )
                nc.any.tensor_copy(vt[i][:, ds(d * 512, 512)], ps[:])
        t2.release()

        # ---- P4: expT[nkv, nq] = exp(sum_do kT[do,nkv] * qT[do,nq]) -----
        et = [L.tile([128, NQ], BF16, name=f"e{i}", tag="et", bufs=8) for i in range(8)]
        for i in range(8):  # nkv 128-tile
            for q in range(4):  # nq 512-chunk
                ps = pm.tile([128, 512], F32, name=f"pse{i}_{q}", tag="mm", bufs=6)
                for j in range(8):  # d_out chunk (contraction)
                    nc.tensor.matmul(
                        ps[:],
                        lhsT=kt[j][:, ds(i * 128, 128)],
                        rhs=qt_all[:, ds(j * NQ + q * 512, 512)],
                        start=(j == 0),
                        stop=(j == 7),
                    )
                nc.scalar.activation(
                    et[i][:, ds(q * 512, 512)],
                    ps[:],
                    mybir.ActivationFunctionType.Exp,
                )

        # ---- P5: Z[nq] = sum_nkv expT[nkv, nq] ---------------------------
        ones = L.tile([128, 1], F32, name="ones", bufs=1)
        nc.vector.memset(ones[:], 1.0)
        # Partial partition-sums on the (otherwise idle) vector engine: a
        # 3-level f32 add-tree collapses the 8 et tiles to one, so the PE
        # only streams 4 ones-matmuls instead of 32.  Z leaves the chip as
        # a plain [1, NQ] row (host divides by it), so no transposes needed.
        t3 = tc.alloc_tile_pool(name="t3", bufs=1)
        s0 = [t3.tile([128, NQ], F32, name=f"es0_{h}", tag="es", bufs=3) for h in range(2)]
        nc.vector.tensor_add(s0[0][:], et[0][:], et[1][:])
        nc.vector.tensor_add(s0[1][:], et[2][:], et[3][:])
        s1 = t3.tile([128, NQ], F32, name="es1", tag="es2", bufs=2)
        nc.vector.tensor_add(s1[:], s0[0][:], s0[1][:])
        s0b = [t3.tile([128, NQ], F32, name=f"es0b_{h}", tag="es", bufs=3) for h in range(2)]
        nc.vector.tensor_add(s0b[0][:], et[4][:], et[5][:])
        nc.vector.tensor_add(s0b[1][:], et[6][:], et[7][:])
        s2 = t3.tile([128, NQ], F32, name="es2", tag="es2", bufs=2)
        nc.vector.tensor_add(s2[:], s0b[0][:], s0b[1][:])
        stot = t3.tile([128, NQ], F32, name="estot", tag="es", bufs=3)
        nc.vector.tensor_add(stot[:], s1[:], s2[:])
        zrow = L.tile([1, NQ], F32, name="zrow", bufs=1)
        for q in range(4):
            psz = pm.tile([1, 512], F32, name=f"psz{q}", tag="z", bufs=2)
            nc.tensor.matmul(
                psz[:],
                lhsT=ones[:],
                rhs=stot[:, ds(q * 512, 512)],
                start=True,
                stop=True,
            )
            nc.any.tensor_copy(zrow[0:1, ds(q * 512, 512)], psz[:])
        nc.sync.dma_start(out=z_d[:], in_=zrow[:])
        t3.release()

        # ---- P7: out'[nq, do] = sum_nkv expT[nkv,nq] * v[nkv,do] --------
        for t in range(16):  # nq 128-tile
            for d in range(2):  # d_out 512-chunk
                ps = pm.tile([128, 512], F32, name=f"pso{t}_{d}", tag="mm", bufs=6)
                for i in range(8):  # nkv contraction
                    nc.tensor.matmul(
                        ps[:],
                        lhsT=et[i][:, ds(t * 128, 128)],
                        rhs=vt[i][:, ds(d * 512, 512)],
                        start=(i == 0),
                        stop=(i == 7),
                    )
                ob = L.tile([128, 512], F32, name=f"o{t}_{d}", tag="o", bufs=3)
                nc.any.tensor_copy(ob[:], ps[:])
                nc.sync.dma_start(
                    out=out_d[ds(t * 128, 128), ds(d * 512, 512)], in_=ob[:]
                )
        pm.release()
        L.release()

    _split_sync_waits(nc)
    return nc


_NC_CACHE = {}


def _get_nc():
    if "nc" not in _NC_CACHE:
        _NC_CACHE["nc"] = build_attention_nc()
    return _NC_CACHE["nc"]


def _pack_pmajor(a):
    """[1024, N] f32 -> bf16 p-major SBUF image [128, 8*N]:
    out[p, c*N + n] = a[c*128 + p, n]."""
    n = a.shape[1]
    return np.ascontiguousarray(
        a.reshape(8, 128, n).transpose(1, 0, 2).reshape(128, 8 * n)
    ).astype(NP_BF16)


def _pack_w_slabs(w):
    """[1024, 1024] f32 weight -> [128, 8192] bf16, slab j at cols
    [j*1024, (j+1)*1024): out[p, j*1024 + c*128 + m] = w[c*128+p, j*128+m]."""
    r = w.reshape(8, 128, 8, 128).transpose(1, 2, 0, 3)  # c,p,j,m -> p,j,c,m
    return np.ascontiguousarray(r.reshape(128, 8192)).astype(NP_BF16)


def make_in_maps(x, y, Wq, Wkv):
    """Host-side sharding + layout prep. Returns in_maps for cores 0-7."""
    scale = DIM ** (-0.5)
    wq_p = _pack_w_slabs(np.asarray(Wq, np.float32) * scale)
    wkv = np.asarray(Wkv, np.float32)
    wk_p = _pack_w_slabs(wkv[:, :DIM])
    wv_p = _pack_pmajor(np.ascontiguousarray(wkv[:, DIM:]))

    # Odd cores compute wq slabs 4-7 (the pair AllGather concatenates
    # even-half then odd-half, so both cores hold slabs in 0..7 order
    # locally as [even slabs, odd slabs]).  wk is permuted per-parity so
    # kt[j] pairs with q slab j: even cores keep 0..7; odd cores ALSO keep
    # 0..7 -- the gathered q is [0-3, 4-7] on BOTH cores, so wk needs NO
    # permutation.  (Kept explicit here for clarity.)
    x = np.asarray(x, np.float32)
    y = np.asarray(y, np.float32)
    in_maps = []
    for core in range(N_CORES):
        b, s = divmod(core, 2)
        xT = _pack_pmajor(np.ascontiguousarray(x[b].T))
        yT = _pack_pmajor(
            np.ascontiguousarray(y[b, s * NKV_SHARD : (s + 1) * NKV_SHARD, :].T)
        )
        if core % 2 == 0:
            wq_half, wk_loc = wq_p[:, : 4 * 1024], wk_p
        else:
            wq_half = wq_p[:, 4 * 1024 :]
            wk_loc = np.concatenate(
                [wk_p[:, 4 * 1024 :], wk_p[:, : 4 * 1024]], axis=1
            )
        pidx = ((core ^ 1) * 128 + np.arange(128, dtype=np.int32)).reshape(128, 1)
        sidx = (core * 128 + np.arange(128, dtype=np.int32)).reshape(128, 1)
        in_maps.append(
            {
                "xt": xT,
                "yt": yT,
                "pidx": pidx,
                "sidx": sidx,
                "wq": np.ascontiguousarray(wq_half),
                "wk0": np.ascontiguousarray(wk_loc[:, :1024]),
                "wk17": np.ascontiguousarray(wk_loc[:, 1024:]),
                "wv": wv_p,
            }
        )
    return in_maps


def run_sharded(x, y, Wq, Wkv, trace=False, tmpdir=None):
    """Run the SPMD kernel; returns (full_output, BassKernelResults)."""
    nc = _get_nc()
    in_maps = make_in_maps(x, y, Wq, Wkv)
    try:
        res = run_bass_kernel_spmd(
            nc, in_maps, core_ids=list(range(N_CORES)), trace=trace, tmpdir=tmpdir
        )
    except Exception:
        # one retry: transient NRT device states (e.g. a previous crashed
        # load) usually clear on the next attempt
        res = run_bass_kernel_spmd(
            nc, in_maps, core_ids=list(range(N_CORES)), trace=trace, tmpdir=tmpdir
        )
    out = np.empty((B, NQ, DIM), np.float32)
    for b in range(B):
        r0, r1 = res.results[2 * b], res.results[2 * b + 1]
        num = r0["out"] + r1["out"]
        z = (r0["zout"] + r1["zout"]).reshape(NQ)
        out[b] = num / z[:, None]
    return out, res


def kernel(x, y, Wq, Wkv):
    out, _ = run_sharded(x, y, Wq, Wkv)
    return out


# revision 29
# speedup vs baseline: 1.0667x; 1.0667x over previous
"""Self-contained Trainium2 Bass kernel for single-head full-dim attention.

Reference computation (fp32 jax):
    q  = x @ Wq                      # [B, Nq, D]
    kv = y @ Wkv                     # [B, Nkv, 2D] -> k, v
    attn = softmax(q * D^-0.5 @ k^T) # [B, Nq, Nkv]
    out  = attn @ v                  # [B, Nq, D]
with B=4, Nq=Nkv=2048, D=1024.

Distribution: data parallel over 8 NeuronCores, shard = (batch b,
kv-half s).  Each core computes q for ALL 2048 queries of its batch
(cheap, duplicated across the pair), K/V for its 1024 keys, the
2048x1024 exp-score block, and the UNNORMALIZED output block
out'_s = exp(S_s) @ v_s plus the partial softmax denominator
Z_s = sum_k exp(S_s).  The host combines the two halves:
out = (out'_0 + out'_1) / (Z_0 + Z_1).  This avoids both collectives
and the (2x more expensive) duplicated K/V compute of a query-sharded
layout.

Layout trick: everything on-chip is computed transposed
([feature, token]) so the TensorEngine can contract along partitions
without any on-chip transposes.  The host pre-transposes and packs each
input into the EXACT [128, N] p-major SBUF image, so every input DMA is
a single fully-dense descriptor (16-32KB contiguous per partition ->
peak HBM read bw; strided 256B-row layouts measured only ~217 GB/s vs
~400 GB/s dense).  Issue order = criticality: wk slabs 0-2, yt (P2's
gate), wk slabs 3-7, wv (P3), xt + wq (P1, needed ~60us in).  The
D^-0.5 scale is folded into Wq; all matmul operands are bf16 (fp32
PSUM accumulation).  Softmax uses exp without max-subtraction (scores
~ N(0,1) by construction; fp32 exp is safe) on the scalar engine; Z is
a ones-vector matmul row, shipped as [1, NQ].
"""

import numpy as np
import ml_dtypes

import concourse.bass as bass
import concourse.mybir as mybir
import concourse.tile as tile
from concourse.bass import ds
from concourse.bass_utils import run_bass_kernel_spmd

DIM = 1024
B = 4
NQ = 2048
NKV = 2048
N_CORES = 8
NKV_SHARD = 1024  # keys per core
WARMUP = 30  # HAM warm-up matmuls; ends ~when yt lands (~16us)

BF16 = mybir.dt.bfloat16
F32 = mybir.dt.float32
NP_BF16 = ml_dtypes.bfloat16


def _split_sync_waits(nc, max_waits: int = 1):
    """walrus in this toolchain rejects instructions carrying more than one
    sem wait ("Too many sync wait commands").  Hoist extra waits onto
    preceding same-engine NOPs: the engine dispatches in order, so waiting
    just before the instruction is semantically identical (at worst it
    delays issue slightly)."""
    import bass_rust as _bass_rust

    for f in nc.m.functions:
        for bb in f.blocks:
            insts = list(bb.instructions)
            out = []
            changed = False
            for inst in insts:
                si = getattr(inst, "sync_info", None)
                waits = list(si.on_wait) if si is not None and si.on_wait else []
                if len(waits) > max_waits:
                    changed = True
                    extra, keep = waits[:-max_waits], waits[-max_waits:]
                    for k in range(0, len(extra), max_waits):
                        nop = mybir.InstNoOp(
                            name=f"{inst.name}_sw{k}", engine=inst.engine,
                            ins=[], outs=[],
                        )
                        nop.sync_info = _bass_rust.SyncInfo(
                            on_wait=extra[k : k + max_waits], on_update=[]
                        )
                        out.append(nop)
                    si.on_wait = keep
                    inst.sync_info = si
                out.append(inst)
            if changed:
                bb.instructions = out


def build_attention_nc():
    """Build the per-core Bass graph (identical on all 8 cores)."""
    nc = bass.Bass()

    # DRAM parameters: host-packed p-major SBUF images (see module doc).
    # wk/wq slab j lives at columns [j*1024, (j+1)*1024), entry (p, j*1024
    # + c*128 + m) = W[c*128 + p, j*128 + m]; yt/xt/wv chunk c at columns
    # [c*N, (c+1)*N), entry (p, c*N + n) = T[c*128 + p, n].
    wk0_d = nc.declare_dram_parameter("wk0", [128, 1024], BF16, isOutput=False)
    yt_d = nc.declare_dram_parameter("yt", [128, 8 * NKV_SHARD], BF16, isOutput=False)
    wk17_d = nc.declare_dram_parameter("wk17", [128, 7 * 1024], BF16, isOutput=False)
    wv_d = nc.declare_dram_parameter("wv", [128, 8 * 1024], BF16, isOutput=False)
    xt_d = nc.declare_dram_parameter("xt", [128, 8 * NQ], BF16, isOutput=False)
    wq_d = nc.declare_dram_parameter("wq", [128, 4 * 1024], BF16, isOutput=False)
    pidx_d = nc.declare_dram_parameter("pidx", [128, 1], mybir.dt.int32, isOutput=False)
    sidx_d = nc.declare_dram_parameter("sidx", [128, 1], mybir.dt.int32, isOutput=False)
    out_d = nc.declare_dram_parameter("out", [NQ, DIM], F32, isOutput=True)
    z_d = nc.declare_dram_parameter("zout", [1, NQ], F32, isOutput=True)

    with tile.TileContext(nc) as tc:
        # Long-lived pool: on-chip intermediates live to the end.
        L = tc.alloc_tile_pool(name="L", bufs=1)
        pm = tc.alloc_tile_pool(name="pm", bufs=1, space="PSUM")
        # Transient input pools, released once consumed (LIFO: t1 first).
        t2 = tc.alloc_tile_pool(name="t2", bufs=1)
        t1 = tc.alloc_tile_pool(name="t1", bufs=1)

        # ---- HAM warm-up: dummy matmuls on a zeroed scratch tile run
        # during the otherwise-idle input-DMA window, flipping the PE clock
        # gate to 8/8 (2.4GHz) before the first real matmul arrives.
        ws = t1.tile([128, 512], BF16, name="warm", tag="warm", bufs=1)
        nc.gpsimd.memset(ws[:], 0.0)
        wps = pm.tile([128, 512], F32, name="wps", tag="z", bufs=2)
        for w in range(WARMUP):
            nc.tensor.matmul(
                wps[:], lhsT=ws[:, 0:128], rhs=ws[:],
                start=(w == 0), stop=(w == WARMUP - 1),
            )

        # ---- Input DMAs: one dense descriptor per tensor, issue order =
        # transfer order = criticality order.
        wq = t1.tile([128, 4, 1024], BF16, name="wq", bufs=1)
        nc.sync.dma_start(out=wq[:], in_=wq_d.rearrange("p (s n) -> p s n", s=4))
        pidx = t1.tile([128, 1], mybir.dt.int32, name="pidx", bufs=1)
        nc.sync.dma_start(out=pidx[:], in_=pidx_d[:])
        sidx = t1.tile([128, 1], mybir.dt.int32, name="sidx", bufs=1)
        nc.sync.dma_start(out=sidx[:], in_=sidx_d[:])
        xt = t1.tile([128, 8, NQ], BF16, name="xt", bufs=1)
        nc.sync.dma_start(out=xt[:], in_=xt_d.rearrange("p (c n) -> p c n", c=8))
        wk0 = t2.tile([128, 1, 1024], BF16, name="wk0", bufs=1)
        nc.sync.dma_start(out=wk0[:], in_=wk0_d.rearrange("p (s n) -> p s n", s=1))
        yt = t2.tile([128, 8, NKV_SHARD], BF16, name="yt", bufs=1)
        nc.sync.dma_start(out=yt[:], in_=yt_d.rearrange("p (c n) -> p c n", c=8))
        wk17 = t2.tile([128, 7, 1024], BF16, name="wk17", bufs=1)
        nc.sync.dma_start(out=wk17[:], in_=wk17_d.rearrange("p (s n) -> p s n", s=7))
        wv = t2.tile([128, 8, 1024], BF16, name="wv", bufs=1)
        nc.sync.dma_start(out=wv[:], in_=wv_d.rearrange("p (c n) -> p c n", c=8))

        # ---- P1 (half): this core computes its 4 local wq slabs; the pair
        # exchanges halves via a pairwise AllGather through DRAM bounce
        # buffers while P3 keeps the PE busy.  Host permutes wk slabs per
        # core parity so kt[j] always matches qt slab j (sum over slabs is
        # order-invariant).
        qt_all = L.tile([128, 8 * NQ], BF16, name="qt_all", bufs=1)
        for j in range(4):  # local d_out slab
            for q in range(4):  # nq 512-chunk
                ps = pm.tile([128, 512], F32, name=f"psq{j}_{q}", tag="mm", bufs=6)
                for c in range(8):  # d_in chunk (contraction)
                    nc.tensor.matmul(
                        ps[:],
                        lhsT=wq[:, j, ds(c * 128, 128)],
                        rhs=xt[:, c, ds(q * 512, 512)],
                        start=(c == 0),
                        stop=(c == 7),
                    )
                nc.vector.tensor_copy(qt_all[:, ds(j * NQ + q * 512, 512)], ps[:])

        # 8-core AllGather with a SHARED output buffer (the >4-core fast
        # path: each core writes its 2MB half once into the chip-shared
        # region, one rendezvous).  Each core then gathers back only its
        # PEER's 128 rows via an indirect DMA whose row indices are a
        # host-supplied per-core input (walrus cannot encode register-offset
        # direct DMAs).  kt is host-permuted to local slab order so the
        # kt[j]*qt[j] pairing stays SPMD-uniform.
        qshare = nc.dram_tensor(
            "qshare", [8 * 128 + 1, 4 * NQ], BF16, kind="Internal",
            addr_space="Shared",
        )
        nc.gpsimd.indirect_dma_start(
            out=qshare[:],
            out_offset=bass.IndirectOffsetOnAxis(ap=sidx[:, :1], axis=0),
            in_=qt_all[:, ds(0, 2 * NQ)],
            in_offset=None,
        )
        nc.gpsimd.indirect_dma_start(
            out=qshare[:],
            out_offset=bass.IndirectOffsetOnAxis(ap=sidx[:, :1], axis=0),
            in_=qt_all[:, ds(2 * NQ, 2 * NQ)],
            in_offset=None,
            element_offset=2 * NQ,
        )
        # 256-byte 8-core AllReduce used purely as a barrier.  Its input
        # chain starts with a probe READ of qshare (conservative shadow dep
        # -> runs after this core's scatter), bounced through a Local DRAM
        # tile (collectives cannot read Shared).  Rank arrival therefore
        # implies that core's scatter completed, so cc-done means every
        # core's half is visible.  The cc writes back INTO qshare (row
        # 1024), so the gather below is shadow-ordered after the cc.
        qd = tc.alloc_tile_pool(name="dram", bufs=1, space="DRAM")
        bar_in = qd.tile([1, 128], BF16, name="bar_in", bufs=1)
        nc.gpsimd.dma_start(out=bar_in[:], in_=qshare[0:1, 0:128])
        nc.gpsimd.collective_compute(
            "AllReduce",
            mybir.AluOpType.add,
            replica_groups=[[0, 1, 2, 3, 4, 5, 6, 7]],
            ins=[bar_in.opt()],
            outs=[qshare[1024:1025, 0:128]],
        )
        nc.gpsimd.indirect_dma_start(
            out=qt_all[:, ds(4 * NQ, 4 * NQ)],
            out_offset=None,
            in_=qshare[:],
            in_offset=bass.IndirectOffsetOnAxis(ap=pidx[:, :1], axis=0),
        )
        qd.release()
        t1.release()

        # ---- P2: kT[do, nkv] = sum_di Wk[di, do] * yT[di, nkv] ----------
        kt = [L.tile([128, NKV_SHARD], BF16, name=f"kt{j}", tag="kt", bufs=8) for j in range(8)]
        for j in range(8):
            wkt = wk0 if j < 1 else wk17
            jj = j if j < 1 else j - 1
            for q in range(2):  # nkv 512-chunk
                ps = pm.tile([128, 512], F32, name=f"psk{j}_{q}", tag="mm", bufs=6)
                for c in range(8):
                    nc.tensor.matmul(
                        ps[:],
                        lhsT=wkt[:, jj, ds(c * 128, 128)],
                        rhs=yt[:, c, ds(q * 512, 512)],
                        start=(c == 0),
                        stop=(c == 7),
                    )
                nc.vector.tensor_copy(kt[j][:, ds(q * 512, 512)], ps[:])

        # ---- P3: v[nkv, do] = sum_di yT[di, nkv] * Wv[di, do] -----------
        vt = [L.tile([128, DIM], BF16, name=f"v{i}", tag="v", bufs=8) for i in range(8)]
        for i in range(8):  # nkv 128-tile
            for d in range(2):  # d_out 512-chunk
                ps = pm.tile([128, 512], F32, name=f"psv{i}_{d}", tag="mm", bufs=6)
                for c in range(8):
                    nc.tensor.matmul(
                        ps[:],
                        lhsT=yt[:, c, ds(i * 128, 128)],
                        rhs=wv[:, c, ds(d * 512, 512)],
                        start=(c == 0),
                        stop=(c == 7),
                    )
                nc.vector.tensor_copy(vt[i][:, ds(d * 512, 512)], ps[:])
        t2.release()

        # ---- P4: expT[nkv, nq] = exp(sum_do kT[do,nkv] * qT[do,nq]) -----
        et = [L.tile([128, NQ], BF16, name=f"e{i}", tag="et", bufs=8) for i in range(8)]
        for i in range(8):  # nkv 128-tile
            for q in range(4):  # nq 512-chunk
                ps = pm.tile([128, 512], F32, name=f"pse{i}_{q}", tag="mm", bufs=6)
                for j in range(8):  # d_out chunk (contraction)
                    nc.tensor.matmul(
                        ps[:],
                        lhsT=kt[j][:, ds(i * 128, 128)],
                        rhs=qt_all[:, ds(j * NQ + q * 512, 512)],
                        start=(j == 0),
                        stop=(j == 7),
                    )
                nc.scalar.activation(
                    et[i][:, ds(q * 512, 512)],
                    ps[:],
                    mybir.ActivationFunctionType.Exp,
                )

        # ---- P5: Z[nq] = sum_nkv expT[nkv, nq] ---------------------------
        ones = L.tile([128, 1], F32, name="ones", bufs=1)
        nc.vector.memset(ones[:], 1.0)
        # Partial partition-sums on the (otherwise idle) vector engine: a
        # 3-level f32 add-tree collapses the 8 et tiles to one, so the PE
        # only streams 4 ones-matmuls instead of 32.  Z leaves the chip as
        # a plain [1, NQ] row (host divides by it), so no transposes needed.
        t3 = tc.alloc_tile_pool(name="t3", bufs=1)
        s0 = [t3.tile([128, NQ], F32, name=f"es0_{h}", tag="es", bufs=3) for h in range(2)]
        nc.vector.tensor_add(s0[0][:], et[0][:], et[1][:])
        nc.vector.tensor_add(s0[1][:], et[2][:], et[3][:])
        s1 = t3.tile([128, NQ], F32, name="es1", tag="es2", bufs=2)
        nc.vector.tensor_add(s1[:], s0[0][:], s0[1][:])
        s0b = [t3.tile([128, NQ], F32, name=f"es0b_{h}", tag="es", bufs=3) for h in range(2)]
        nc.vector.tensor_add(s0b[0][:], et[4][:], et[5][:])
        nc.vector.tensor_add(s0b[1][:], et[6][:], et[7][:])
        s2 = t3.tile([128, NQ], F32, name="es2", tag="es2", bufs=2)
        nc.vector.tensor_add(s2[:], s0b[0][:], s0b[1][:])
        stot = t3.tile([128, NQ], F32, name="estot", tag="es", bufs=3)
        nc.vector.tensor_add(stot[:], s1[:], s2[:])
        zrow = L.tile([1, NQ], F32, name="zrow", bufs=1)
        for q in range(4):
            psz = pm.tile([1, 512], F32, name=f"psz{q}", tag="z", bufs=2)
            nc.tensor.matmul(
                psz[:],
                lhsT=ones[:],
                rhs=stot[:, ds(q * 512, 512)],
                start=True,
                stop=True,
            )
            nc.vector.tensor_copy(zrow[0:1, ds(q * 512, 512)], psz[:])
        nc.sync.dma_start(out=z_d[:], in_=zrow[:])
        t3.release()

        # ---- P7: out'[nq, do] = sum_nkv expT[nkv,nq] * v[nkv,do] --------
        for t in range(16):  # nq 128-tile
            for d in range(2):  # d_out 512-chunk
                ps = pm.tile([128, 512], F32, name=f"pso{t}_{d}", tag="mm", bufs=6)
                for i in range(8):  # nkv contraction
                    nc.tensor.matmul(
                        ps[:],
                        lhsT=et[i][:, ds(t * 128, 128)],
                        rhs=vt[i][:, ds(d * 512, 512)],
                        start=(i == 0),
                        stop=(i == 7),
                    )
                ob = L.tile([128, 512], F32, name=f"o{t}_{d}", tag="o", bufs=3)
                nc.vector.tensor_copy(ob[:], ps[:])
                nc.sync.dma_start(
                    out=out_d[ds(t * 128, 128), ds(d * 512, 512)], in_=ob[:]
                )
        pm.release()
        L.release()

    _split_sync_waits(nc)
    return nc


_NC_CACHE = {}


def _get_nc():
    if "nc" not in _NC_CACHE:
        _NC_CACHE["nc"] = build_attention_nc()
    return _NC_CACHE["nc"]


def _pack_pmajor(a):
    """[1024, N] f32 -> bf16 p-major SBUF image [128, 8*N]:
    out[p, c*N + n] = a[c*128 + p, n]."""
    n = a.shape[1]
    return np.ascontiguousarray(
        a.reshape(8, 128, n).transpose(1, 0, 2).reshape(128, 8 * n)
    ).astype(NP_BF16)


def _pack_w_slabs(w):
    """[1024, 1024] f32 weight -> [128, 8192] bf16, slab j at cols
    [j*1024, (j+1)*1024): out[p, j*1024 + c*128 + m] = w[c*128+p, j*128+m]."""
    r = w.reshape(8, 128, 8, 128).transpose(1, 2, 0, 3)  # c,p,j,m -> p,j,c,m
    return np.ascontiguousarray(r.reshape(128, 8192)).astype(NP_BF16)


def make_in_maps(x, y, Wq, Wkv):
    """Host-side sharding + layout prep. Returns in_maps for cores 0-7."""
    scale = DIM ** (-0.5)
    wq_p = _pack_w_slabs(np.asarray(Wq, np.float32) * scale)
    wkv = np.asarray(Wkv, np.float32)
    wk_p = _pack_w_slabs(wkv[:, :DIM])
    wv_p = _pack_pmajor(np.ascontiguousarray(wkv[:, DIM:]))

    # Odd cores compute wq slabs 4-7 (the pair AllGather concatenates
    # even-half then odd-half, so both cores hold slabs in 0..7 order
    # locally as [even slabs, odd slabs]).  wk is permuted per-parity so
    # kt[j] pairs with q slab j: even cores keep 0..7; odd cores ALSO keep
    # 0..7 -- the gathered q is [0-3, 4-7] on BOTH cores, so wk needs NO
    # permutation.  (Kept explicit here for clarity.)
    x = np.asarray(x, np.float32)
    y = np.asarray(y, np.float32)
    in_maps = []
    for core in range(N_CORES):
        b, s = divmod(core, 2)
        xT = _pack_pmajor(np.ascontiguousarray(x[b].T))
        yT = _pack_pmajor(
            np.ascontiguousarray(y[b, s * NKV_SHARD : (s + 1) * NKV_SHARD, :].T)
        )
        if core % 2 == 0:
            wq_half, wk_loc = wq_p[:, : 4 * 1024], wk_p
        else:
            wq_half = wq_p[:, 4 * 1024 :]
            wk_loc = np.concatenate(
                [wk_p[:, 4 * 1024 :], wk_p[:, : 4 * 1024]], axis=1
            )
        pidx = ((core ^ 1) * 128 + np.arange(128, dtype=np.int32)).reshape(128, 1)
        sidx = (core * 128 + np.arange(128, dtype=np.int32)).reshape(128, 1)
        in_maps.append(
            {
                "xt": xT,
                "yt": yT,
                "pidx": pidx,
                "sidx": sidx,
                "wq": np.ascontiguousarray(wq_half),
                "wk0": np.ascontiguousarray(wk_loc[:, :1024]),
                "wk17": np.ascontiguousarray(wk_loc[:, 1024:]),
                "wv": wv_p,
            }
        )
    return in_maps


def run_sharded(x, y, Wq, Wkv, trace=False, tmpdir=None):
    """Run the SPMD kernel; returns (full_output, BassKernelResults)."""
    nc = _get_nc()
    in_maps = make_in_maps(x, y, Wq, Wkv)
    try:
        res = run_bass_kernel_spmd(
            nc, in_maps, core_ids=list(range(N_CORES)), trace=trace, tmpdir=tmpdir
        )
    except Exception:
        # one retry: transient NRT device states (e.g. a previous crashed
        # load) usually clear on the next attempt
        res = run_bass_kernel_spmd(
            nc, in_maps, core_ids=list(range(N_CORES)), trace=trace, tmpdir=tmpdir
        )
    out = np.empty((B, NQ, DIM), np.float32)
    for b in range(B):
        r0, r1 = res.results[2 * b], res.results[2 * b + 1]
        num = r0["out"] + r1["out"]
        z = (r0["zout"] + r1["zout"]).reshape(NQ)
        out[b] = num / z[:, None]
    return out, res


def kernel(x, y, Wq, Wkv):
    out, _ = run_sharded(x, y, Wq, Wkv)
    return out
